# revision 1
# baseline (speedup 1.0000x reference)
"""Causal self-attention (B=8, T=1024, C=1024, H=16) on 8 trn2 NeuronCores.

Data-parallel over batch: each core computes one batch element's full
attention layer; no collectives. All matmuls in fp32r (fp32 storage,
1 cycle/row at free-dim >= 256, ~1.6e-4 relative error per matmul).

Host pre-transposes inputs so every contraction dim lands on partitions:
  xT    [C, T]        x[b].T
  wqk   [C, 8, 256]   w_attn[:2C].T, k/q packed per head-pair
  wvT   [C, C]        w_attn[2C:].T
  wpT   [C, C]        w_proj.T
  bb    [128, C]      b_proj broadcast along partitions
  maskT [128, 128]    additive causal mask in [j, i] orientation

Per-core pipeline:
  V-proj  v[t, vj] = xT.T @ wvT, staged as v_aug tiles [t, pair, v|1|v|1]
  per pair m: K/Q-proj -> kT/qT [feat, t]; the two heads' QK^T matmuls
    are interleaved (even head on PE rows 0:64, odd on 64:128 -> they run
    concurrently in separate row groups); +mask on diagonal block;
    exp(s/8) -> pT (no max-subtraction: logits are ~N(0, 0.17));
    AV per head: yT_aug[65, i] += v_aug.T @ pT (row 64 = softmax denom
    via the ones column); psum evicted immediately to yr; denominator
    row is reshaped to [128, 8] via a DRAM hop so reciprocal runs wide,
    then partition-broadcast back; yT = yr * recip -> YT pair tiles
  out[t, m] = YT.T @ wpT + b
"""
import sys
from contextlib import ExitStack

sys.path.insert(0, "/opt/trn_rl_repo")
import numpy as np

from concourse import bacc, mybir
from concourse import tile
from concourse.bass_utils import run_bass_kernel_spmd

B, T, C = 8, 1024, 1024
H = 16
D = C // H  # 64
NCORES = 8
NPAIR = H // 2  # 8
NTB = T // 128  # 8
NCB = C // 128  # 8
F32 = mybir.dt.float32
F32R = mybir.dt.float32r
AF = mybir.ActivationFunctionType
SCALE = 1.0 / 8.0  # 1/sqrt(D)
NEG = -1.0e30


def r(ap):
    return ap.bitcast(F32R)


def build():
    nc = bacc.Bacc(target_bir_lowering=False)
    xT = nc.dram_tensor("xT", [C, T], F32, kind="ExternalInput")
    wqk = nc.dram_tensor("wqk", [C, NPAIR, 256], F32, kind="ExternalInput")
    wvT = nc.dram_tensor("wvT", [C, C], F32, kind="ExternalInput")
    wpT = nc.dram_tensor("wpT", [C, C], F32, kind="ExternalInput")
    bb = nc.dram_tensor("bb", [128, C], F32, kind="ExternalInput")
    maskT = nc.dram_tensor("maskT", [128, 128], F32, kind="ExternalInput")
    ones = nc.dram_tensor("ones", [128, H, 1], F32, kind="ExternalInput")
    out = nc.dram_tensor("out", [T, C], F32, kind="ExternalOutput")

    with tile.TileContext(nc) as tc, ExitStack() as top:
        const = top.enter_context(tc.tile_pool(name="const", bufs=1))
        ytp = top.enter_context(tc.tile_pool(name="yt", bufs=1))
        smp = top.enter_context(tc.tile_pool(name="sm", bufs=1))
        psa = top.enter_context(tc.tile_pool(name="psa", bufs=2, space="PSUM"))
        psb = top.enter_context(tc.tile_pool(name="psb", bufs=2, space="PSUM"))
        dramp = top.enter_context(tc.tile_pool(name="dram", bufs=1, space="DRAM"))

        pstat = const.tile([128, 128], F32R, name="pstat")
        nc.scalar.dma_start(out=pstat[:], in_=r(xT[0:128, 0:128]))
        pmov = const.tile([128, 512], F32R, name="pmov")
        nc.scalar.dma_start(out=pmov[:], in_=r(xT[0:128, 0:512]))
        bbt = const.tile([128, C], F32, name="bbt")
        nc.sync.dma_start(out=bbt[:], in_=bb[:])
        mkt = const.tile([128, 128], F32, name="mkt")
        nc.sync.dma_start(out=mkt[:], in_=maskT[:])

        # warm-up primer: keep the PE busy (and the HAM un-throttled)
        # while the initial x/w DMAs land. Results are never read.
        prim = psa.tile([128, 512], F32, name="prim", tag="a", bufs=2)
        for _ in range(24):
            nc.tensor.matmul(prim[:, 0:512], pstat[:], pmov[:],
                             start=True, stop=True)

        yts = []
        with ExitStack() as mid:
            xp = mid.enter_context(tc.tile_pool(name="xp", bufs=1))
            vtp = mid.enter_context(tc.tile_pool(name="vt", bufs=1))
            xts = []
            for cb in range(NCB):
                xt = xp.tile([128, T], F32R, name=f"x{cb}", tag="x", bufs=NCB)
                nc.scalar.dma_start(out=xt[:],
                                    in_=r(xT[cb * 128:(cb + 1) * 128, :]))
                xts.append(xt)

            # ---- V projection ----
            vts = []
            with tc.tile_pool(name="wv", bufs=1) as wv:
                wvt = wv.tile([128, NCB, C], F32R, name="wvt")
                for cb in range(NCB):
                    nc.sync.dma_start(
                        out=wvt[:, cb, :],
                        in_=r(wvT[cb * 128:(cb + 1) * 128, :]))
                for tb in range(NTB):
                    vt = vtp.tile([128, NPAIR, 130], F32R, name=f"v{tb}",
                                  tag="v", bufs=NTB)
                    for half in range(2):
                        ps = psb.tile([128, 1024], F32, name="vps", tag="att",
                                      bufs=2)
                        for cb in range(NCB):
                            nc.tensor.matmul(
                                ps[:, 0:512],
                                xts[cb][:, tb * 128:(tb + 1) * 128],
                                wvt[:, cb, half * 512:(half + 1) * 512],
                                start=(cb == 0), stop=(cb == NCB - 1))
                        psv = ps[:, 0:512].rearrange("p (pr f) -> p pr f",
                                                     f=128)
                        nc.vector.tensor_copy(
                            vt[:, half * 4:(half + 1) * 4, 0:64],
                            psv[:, :, 0:64])
                        nc.vector.tensor_copy(
                            vt[:, half * 4:(half + 1) * 4, 65:129],
                            psv[:, :, 64:128])
                    vt2 = vt.rearrange("p pr (two f) -> p (pr two) f", f=65)
                    nc.scalar.dma_start(out=vt2[:, :, 64:65], in_=r(ones[:]))
                    vts.append(vt)

            # ---- attention, per head pair ----
            with ExitStack() as att_stack:
                wq = att_stack.enter_context(tc.tile_pool(name="wq", bufs=1))
                kqp = att_stack.enter_context(tc.tile_pool(name="kq", bufs=1))
                ptp = att_stack.enter_context(tc.tile_pool(name="pt", bufs=1))
                wqms = {}

                def load_wqm(mm_):
                    wqm_ = wq.tile([128, NCB, 256], F32R, name=f"wqm{mm_}",
                                   tag="wqm", bufs=2)
                    nc.scalar.dma_start(
                        out=wqm_[:],
                        in_=r(wqk[:, mm_, :].rearrange("(cb p) f -> p cb f",
                                                       p=128)))
                    wqms[mm_] = wqm_

                kqs = {}

                def kqproj(mm_):
                    wqm_ = wqms[mm_]
                    km_ = kqp.tile([128, T], F32R, name=f"k{mm_}", tag="km",
                                   bufs=2)
                    qm_ = kqp.tile([128, T], F32R, name=f"q{mm_}", tag="qm",
                                   bufs=2)
                    for kq in range(2):  # k pass, q pass
                        dst = km_ if kq == 0 else qm_
                        for th in range(2):
                            ps = psa.tile([128, 512], F32, name="kqps",
                                          tag="a", bufs=2)
                            for cb in range(NCB):
                                nc.tensor.matmul(
                                    ps[:],
                                    wqm_[:, cb, kq * 128:(kq + 1) * 128],
                                    xts[cb][:, th * 512:(th + 1) * 512],
                                    start=(cb == 0), stop=(cb == NCB - 1))
                            nc.vector.tensor_copy(
                                dst[:, th * 512:(th + 1) * 512], ps[:])
                    kqs[mm_] = (km_, qm_)

                load_wqm(0)
                load_wqm(1)
                kqproj(0)
                for m in range(NPAIR):
                    km, qm = kqs[m]
                    yt = ytp.tile([128, T], F32R, name=f"yt{m}", tag="yt",
                                  bufs=NPAIR)
                    yts.append(yt)

                    # QK^T interleaved across the two heads: even head in PE
                    # rows 0:64, odd head in rows 64:128 -> concurrent.
                    pts = {0: [], 1: []}
                    for jb in range(NTB):
                        w = T - jb * 128
                        for hp in range(2):
                            p0 = hp * 64
                            pt = ptp.tile([128, w], F32R,
                                          name=f"pt{jb}_{hp}",
                                          tag=f"pt{jb}", bufs=2)
                            pts[hp].append(pt)
                            att = psb.tile([128, 1024], F32, name="att",
                                           tag="att", bufs=2)
                            for ch in range(2):
                                i0 = max(jb * 128, ch * 512)
                                cw = (ch + 1) * 512 - i0
                                if cw <= 0:
                                    continue
                                nc.tensor.matmul(
                                    att[:, i0:i0 + cw],
                                    km[p0:p0 + 64,
                                       jb * 128:(jb + 1) * 128],
                                    qm[p0:p0 + 64, i0:i0 + cw],
                                    start=True, stop=True)
                            nc.vector.tensor_add(
                                att[:, jb * 128:jb * 128 + 128],
                                att[:, jb * 128:jb * 128 + 128], mkt[:])
                            nc.scalar.activation(
                                pt[:, 0:w], att[:, jb * 128:T], AF.Exp,
                                scale=SCALE)

                    if m + 1 < NPAIR:
                        kqproj(m + 1)  # PE filler while exps drain

                    for hp in range(2):  # AV + softmax denom per head
                        voff = 65 * hp
                        ya = [psb.tile([128, 512], F32, name="ya",
                                       tag="ya", bufs=2) for _ in range(2)]
                        for jb in range(NTB):
                            for ch in range(2):
                                if jb * 128 >= (ch + 1) * 512:
                                    continue
                                i0 = max(jb * 128, ch * 512)
                                cw = (ch + 1) * 512 - i0
                                first = (jb == 0)
                                last = (jb == NTB - 1) or \
                                    (ch == 0 and jb == 3)
                                nc.tensor.matmul(
                                    ya[ch][0:65,
                                           i0 - ch * 512:i0 - ch * 512 + cw],
                                    vts[jb][:, m, voff:voff + 65],
                                    pts[hp][jb][:, i0 - jb * 128:
                                                i0 - jb * 128 + cw],
                                    start=first, stop=last)
                        # evict psum fast: yr rows 0:64 = raw y, row 64 = denom
                        yr = smp.tile([65, T], F32, name="yr", tag="yr",
                                      bufs=3)
                        for ch in range(2):
                            nc.vector.tensor_copy(
                                yr[0:65, ch * 512:(ch + 1) * 512],
                                ya[ch][0:65, 0:512])
                        # denom -> DRAM -> [128, 8] -> recip -> DRAM -> bcast
                        dd0 = dramp.tile([1, T], F32, name="dd0", tag="dd0",
                                         bufs=2)
                        nc.sync.dma_start(out=dd0[:], in_=yr[64:65, :])
                        dtr = smp.tile([128, 8], F32, name="dtr", tag="dtr",
                                       bufs=2)
                        nc.sync.dma_start(
                            out=dtr[:],
                            in_=dd0[0, :].rearrange("(p q) -> p q", q=8))
                        rtr = smp.tile([128, 8], F32, name="rtr", tag="rtr",
                                       bufs=2)
                        nc.vector.reciprocal(rtr[:], dtr[:])
                        dd = dramp.tile([1, T], F32, name="dd", tag="dd",
                                        bufs=2)
                        nc.sync.dma_start(
                            out=dd[0, :].rearrange("(p q) -> p q", q=8),
                            in_=rtr[:])
                        bc = smp.tile([64, T], F32, name="bc", tag="bc",
                                      bufs=3)
                        nc.sync.dma_start(
                            out=bc[:], in_=dd[0, :].partition_broadcast(64))
                        # normalize into YT pair tile
                        if hp == 0:
                            nc.vector.tensor_mul(yt[0:64, :], yr[0:64, :],
                                                 bc[:])
                        else:
                            ytmp = smp.tile([64, T], F32R, name="ytmp",
                                            tag="ytmp", bufs=1)
                            nc.vector.tensor_mul(ytmp[:], yr[0:64, :], bc[:])
                            nc.sync.dma_start(out=yt[64:128, :], in_=ytmp[:])
                    if m + 2 < NPAIR:
                        load_wqm(m + 2)

        # ---- output projection ----
        with tc.tile_pool(name="wp", bufs=1) as wp, \
             tc.tile_pool(name="os", bufs=2) as osp:
            wpt = wp.tile([128, NCB, C], F32R, name="wpt")
            nc.scalar.dma_start(
                out=wpt[:],
                in_=r(wpT[:].rearrange("(cb p) j -> p cb j", p=128)))
            for tb in range(NTB):
                ost = osp.tile([128, C], F32, name="ost", tag="ost", bufs=2)
                for half in range(2):
                    ps = psa.tile([128, 512], F32, name="pps", tag="a",
                                  bufs=2)
                    for m in range(NPAIR):
                        nc.tensor.matmul(
                            ps[:], yts[m][:, tb * 128:(tb + 1) * 128],
                            wpt[:, m, half * 512:(half + 1) * 512],
                            start=(m == 0), stop=(m == NPAIR - 1))
                    nc.vector.tensor_add(
                        ost[:, half * 512:(half + 1) * 512], ps[:],
                        bbt[:, half * 512:(half + 1) * 512])
                nc.sync.dma_start(
                    out=out[tb * 128:(tb + 1) * 128, :], in_=ost[:])

    nc.compile()
    return nc


_NC = None


def _get_nc():
    global _NC
    if _NC is None:
        _NC = build()
    return _NC


def prep_inputs(x, w_attn, w_proj, b_proj):
    x = np.asarray(x, dtype=np.float32)
    w_attn = np.asarray(w_attn, dtype=np.float32)
    w_proj = np.asarray(w_proj, dtype=np.float32)
    b_proj = np.asarray(b_proj, dtype=np.float32)
    ki = np.ascontiguousarray(w_attn[0:C].T).reshape(C, NPAIR, 128)
    qi = np.ascontiguousarray(w_attn[C:2 * C].T).reshape(C, NPAIR, 128)
    wqkv = np.ascontiguousarray(np.concatenate([ki, qi], axis=2))
    wvTv = np.ascontiguousarray(w_attn[2 * C:3 * C].T)
    wpTv = np.ascontiguousarray(w_proj.T)
    bbv = np.broadcast_to(b_proj, (128, C)).copy()
    ii = np.arange(128)
    mk = np.where(ii[None, :] >= ii[:, None], 0.0, NEG).astype(np.float32)
    shared = {"wqk": wqkv, "wvT": wvTv, "wpT": wpTv, "bb": bbv, "maskT": mk,
              "ones": np.ones((128, H, 1), dtype=np.float32)}
    in_maps = []
    for b in range(B):
        im = dict(shared)
        im["xT"] = np.ascontiguousarray(x[b].T)
        in_maps.append(im)
    return in_maps


def kernel(x, w_attn, w_proj, b_proj):
    nc = _get_nc()
    in_maps = prep_inputs(x, w_attn, w_proj, b_proj)
    res = run_bass_kernel_spmd(nc, in_maps, core_ids=list(range(NCORES)))
    return np.stack([res.results[b]["out"] for b in range(B)]).astype(np.float32)



# revision 13
# speedup vs baseline: 2.5770x; 2.5770x over previous
"""Causal self-attention (B=8, T=1024, C=1024, H=16) on 8 trn2 NeuronCores.

Data-parallel over batch: each core computes one batch element's full
attention layer; no collectives. All matmuls in fp32r (fp32 storage,
1 cycle/row at free-dim >= 256, ~1.6e-4 relative error per matmul).

Host pre-transposes inputs so every contraction dim lands on partitions:
  xT    [C, T]        x[b].T
  wqk   [C, 8, 256]   w_attn[:2C].T, k/q packed per head-pair
  wvT   [C, C]        w_attn[2C:].T
  wpT   [C, C]        w_proj.T
  bb    [128, C]      b_proj broadcast along partitions
  maskT [128, 128]    additive causal mask in [j, i] orientation

Per-core pipeline:
  V-proj  v[t, vj] = xT.T @ wvT, staged as v_aug tiles [t, pair, v|1|v|1]
  per pair m: K/Q-proj -> kT/qT [feat, t]; the two heads' QK^T matmuls
    are interleaved (even head on PE rows 0:64, odd on 64:128 -> they run
    concurrently in separate row groups); +mask on diagonal block;
    exp(s/8) -> pT (no max-subtraction: logits are ~N(0, 0.17));
    AV per head: yT_aug[65, i] += v_aug.T @ pT (row 64 = softmax denom
    via the ones column); psum evicted immediately to yr; denominator
    row is reshaped to [128, 8] via a DRAM hop so reciprocal runs wide,
    then partition-broadcast back; yT = yr * recip -> YT pair tiles
  out[t, m] = YT.T @ wpT + b
"""
import sys
from contextlib import ExitStack

sys.path.insert(0, "/opt/trn_rl_repo")
import numpy as np

from concourse import bacc, mybir
from concourse import tile
from concourse.bass_utils import run_bass_kernel_spmd

B, T, C = 8, 1024, 1024
H = 16
D = C // H  # 64
NCORES = 8
NPAIR = H // 2  # 8
NTB = T // 128  # 8
NCB = C // 128  # 8
F32 = mybir.dt.float32
F32R = mybir.dt.float32r
BF16 = mybir.dt.bfloat16
AF = mybir.ActivationFunctionType
SCALE = 1.0 / 8.0  # 1/sqrt(D)
NEG = -1.0e30


def r(ap):
    return ap.bitcast(F32R)


def build():
    nc = bacc.Bacc(target_bir_lowering=False)
    xT = nc.dram_tensor("xT", [C, T], F32, kind="ExternalInput")
    wqk = nc.dram_tensor("wqk", [C, NPAIR, 256], F32, kind="ExternalInput")
    wvT = nc.dram_tensor("wvT", [C, C], F32, kind="ExternalInput")
    wpT = nc.dram_tensor("wpT", [C, C], F32, kind="ExternalInput")
    bb = nc.dram_tensor("bb", [128, C], F32, kind="ExternalInput")
    maskT = nc.dram_tensor("maskT", [128, 128], F32, kind="ExternalInput")
    out = nc.dram_tensor("out", [T, C], F32, kind="ExternalOutput")

    with tile.TileContext(nc) as tc, ExitStack() as top:
        const = top.enter_context(tc.tile_pool(name="const", bufs=1))
        ytp = top.enter_context(tc.tile_pool(name="yt", bufs=1))
        smp = top.enter_context(tc.tile_pool(name="sm", bufs=1))
        psa = top.enter_context(tc.tile_pool(name="psa", bufs=2, space="PSUM"))
        psb = top.enter_context(tc.tile_pool(name="psb", bufs=2, space="PSUM"))
        dramp = top.enter_context(tc.tile_pool(name="dram", bufs=1, space="DRAM"))

        pstat = const.tile([128, 128], F32R, name="pstat")
        nc.scalar.dma_start(out=pstat[:], in_=r(xT[0:128, 0:128]))
        pmov = const.tile([128, 512], F32R, name="pmov")
        nc.scalar.dma_start(out=pmov[:], in_=r(xT[0:128, 0:512]))
        bbt = const.tile([128, C], F32, name="bbt")
        nc.sync.dma_start(out=bbt[:], in_=bb[:])
        mkt = const.tile([128, 128], F32, name="mkt")
        nc.sync.dma_start(out=mkt[:], in_=maskT[:])
        # prefetch output-proj weights on the (otherwise idle) gpsimd DMA
        # queue so the final phase never waits on HBM
        wpt = const.tile([128, NCB, C], F32R, name="wpt")
        nc.gpsimd.dma_start(
            out=wpt[:],
            in_=r(wpT[:].rearrange("(cb p) j -> p cb j", p=128)))

        # warm-up primer: keep the PE busy (and the HAM un-throttled)
        # while the initial x/w DMAs land. Results are never read.
        prim = psa.tile([128, 512], F32, name="prim", tag="a", bufs=2)
        for _ in range(24):
            nc.tensor.matmul(prim[:, 0:512], pstat[:], pmov[:],
                             start=True, stop=True)

        yts = []
        with ExitStack() as mid:
            xp = mid.enter_context(tc.tile_pool(name="xp", bufs=1))
            vtp = mid.enter_context(tc.tile_pool(name="vt", bufs=1))
            xts = []
            for cb in range(NCB):
                xt = xp.tile([128, T], F32R, name=f"x{cb}", tag="x", bufs=NCB)
                nc.scalar.dma_start(out=xt[:],
                                    in_=r(xT[cb * 128:(cb + 1) * 128, :]))
                xts.append(xt)

            # ---- V projection ----
            vts = []
            with tc.tile_pool(name="wv", bufs=1) as wv:
                wvt = wv.tile([128, NCB, C], F32R, name="wvt")
                for cb in range(NCB):
                    nc.sync.dma_start(
                        out=wvt[:, cb, :],
                        in_=r(wvT[cb * 128:(cb + 1) * 128, :]))
                for tb in range(NTB):
                    vt = vtp.tile([128, NPAIR, 130], BF16, name=f"v{tb}",
                                  tag="v", bufs=NTB)
                    for half in range(2):
                        ps = psb.tile([128, 1024], F32, name="vps", tag="att",
                                      bufs=2)
                        for cb in range(NCB):
                            nc.tensor.matmul(
                                ps[:, 0:512],
                                xts[cb][:, tb * 128:(tb + 1) * 128],
                                wvt[:, cb, half * 512:(half + 1) * 512],
                                start=(cb == 0), stop=(cb == NCB - 1))
                        psv = ps[:, 0:512].rearrange("p (pr f) -> p pr f",
                                                     f=128)
                        nc.vector.tensor_copy(
                            vt[:, half * 4:(half + 1) * 4, 0:64],
                            psv[:, :, 0:64])
                        nc.vector.tensor_copy(
                            vt[:, half * 4:(half + 1) * 4, 65:129],
                            psv[:, :, 64:128])
                    vt2 = vt.rearrange("p pr (two f) -> p (pr two) f", f=65)
                    nc.vector.memset(vt2[:, :, 64:65], 1.0)
                    vts.append(vt)

            # ---- attention, per head pair ----
            with ExitStack() as att_stack:
                wq = att_stack.enter_context(tc.tile_pool(name="wq", bufs=1))
                kqp = att_stack.enter_context(tc.tile_pool(name="kq", bufs=1))
                ptp = att_stack.enter_context(tc.tile_pool(name="pt", bufs=1))
                wqms = {}

                def load_wqm(mm_):
                    wqm_ = wq.tile([128, NCB, 256], F32R, name=f"wqm{mm_}",
                                   tag="wqm", bufs=2)
                    nc.scalar.dma_start(
                        out=wqm_[:],
                        in_=r(wqk[:, mm_, :].rearrange("(cb p) f -> p cb f",
                                                       p=128)))
                    wqms[mm_] = wqm_

                kqs = {}

                def kqproj(mm_):
                    wqm_ = wqms[mm_]
                    km_ = kqp.tile([128, T], BF16, name=f"k{mm_}", tag="km",
                                   bufs=2)
                    qm_ = kqp.tile([128, T], BF16, name=f"q{mm_}", tag="qm",
                                   bufs=2)
                    for kq in range(2):  # k pass, q pass
                        dst = km_ if kq == 0 else qm_
                        for th in range(2):
                            ps = psa.tile([128, 512], F32, name="kqps",
                                          tag="a", bufs=2)
                            for cb in range(NCB):
                                nc.tensor.matmul(
                                    ps[:],
                                    wqm_[:, cb, kq * 128:(kq + 1) * 128],
                                    xts[cb][:, th * 512:(th + 1) * 512],
                                    start=(cb == 0), stop=(cb == NCB - 1))
                            nc.vector.tensor_copy(
                                dst[:, th * 512:(th + 1) * 512], ps[:])
                    kqs[mm_] = (km_, qm_)

                load_wqm(0)
                load_wqm(1)
                kqproj(0)
                for m in range(NPAIR):
                    km, qm = kqs[m]
                    yt = ytp.tile([128, T], F32R, name=f"yt{m}", tag="yt",
                                  bufs=NPAIR)
                    yts.append(yt)

                    # QK^T interleaved across the two heads: even head in PE
                    # rows 0:64, odd head in rows 64:128 -> concurrent.
                    pts = {0: [], 1: []}
                    for jb in range(NTB):
                        w = T - jb * 128
                        for hp in range(2):
                            p0 = hp * 64
                            pt = ptp.tile([128, w], BF16,
                                          name=f"pt{jb}_{hp}",
                                          tag=f"pt{jb}", bufs=2)
                            pts[hp].append(pt)
                            att = psb.tile([128, 1024], F32, name="att",
                                           tag="att", bufs=2)
                            for ch in range(2):
                                i0 = max(jb * 128, ch * 512)
                                cw = (ch + 1) * 512 - i0
                                if cw <= 0:
                                    continue
                                nc.tensor.matmul(
                                    att[:, i0:i0 + cw],
                                    km[p0:p0 + 64,
                                       jb * 128:(jb + 1) * 128],
                                    qm[p0:p0 + 64, i0:i0 + cw],
                                    start=True, stop=True)
                            nc.vector.tensor_add(
                                att[:, jb * 128:jb * 128 + 128],
                                att[:, jb * 128:jb * 128 + 128], mkt[:])
                            nc.scalar.activation(
                                pt[:, 0:w], att[:, jb * 128:T], AF.Exp,
                                scale=SCALE)

                    if m + 1 < NPAIR:
                        kqproj(m + 1)  # PE filler while exps drain

                    for hp in range(2):  # AV + softmax denom per head
                        voff = 65 * hp
                        ya = [psb.tile([128, 512], F32, name="ya",
                                       tag="ya", bufs=2) for _ in range(2)]
                        for jb in range(NTB):
                            for ch in range(2):
                                if jb * 128 >= (ch + 1) * 512:
                                    continue
                                i0 = max(jb * 128, ch * 512)
                                cw = (ch + 1) * 512 - i0
                                first = (jb == 0)
                                last = (jb == NTB - 1) or \
                                    (ch == 0 and jb == 3)
                                nc.tensor.matmul(
                                    ya[ch][0:65,
                                           i0 - ch * 512:i0 - ch * 512 + cw],
                                    vts[jb][:, m, voff:voff + 65],
                                    pts[hp][jb][:, i0 - jb * 128:
                                                i0 - jb * 128 + cw],
                                    start=first, stop=last)
                        # evict psum fast: yr rows 0:64 = raw y, row 64 = denom
                        yr = smp.tile([65, T], F32, name="yr", tag="yr",
                                      bufs=3)
                        for ch in range(2):
                            nc.vector.tensor_copy(
                                yr[0:65, ch * 512:(ch + 1) * 512],
                                ya[ch][0:65, 0:512])
                        # denom -> DRAM -> [128, 8] -> recip -> DRAM -> bcast
                        dd0 = dramp.tile([1, T], F32, name="dd0", tag="dd0",
                                         bufs=2)
                        nc.sync.dma_start(out=dd0[:], in_=yr[64:65, :])
                        dtr = smp.tile([128, 8], F32, name="dtr", tag="dtr",
                                       bufs=2)
                        nc.sync.dma_start(
                            out=dtr[:],
                            in_=dd0[0, :].rearrange("(p q) -> p q", q=8))
                        rtr = smp.tile([128, 8], F32, name="rtr", tag="rtr",
                                       bufs=2)
                        nc.vector.reciprocal(rtr[:], dtr[:])
                        dd = dramp.tile([1, T], F32, name="dd", tag="dd",
                                        bufs=2)
                        nc.sync.dma_start(
                            out=dd[0, :].rearrange("(p q) -> p q", q=8),
                            in_=rtr[:])
                        bc = smp.tile([64, T], F32, name="bc", tag="bc",
                                      bufs=3)
                        nc.sync.dma_start(
                            out=bc[:], in_=dd[0, :].partition_broadcast(64))
                        # normalize into YT pair tile
                        if hp == 0:
                            nc.vector.tensor_mul(yt[0:64, :], yr[0:64, :],
                                                 bc[:])
                        else:
                            ytmp = smp.tile([64, T], F32R, name="ytmp",
                                            tag="ytmp", bufs=1)
                            nc.vector.tensor_mul(ytmp[:], yr[0:64, :],
                                                 bc[:])
                            nc.sync.dma_start(out=yt[64:128, :], in_=ytmp[:])
                    if m + 2 < NPAIR:
                        load_wqm(m + 2)

        # ---- output projection ----
        with tc.tile_pool(name="os", bufs=2) as osp:
            for tb in range(NTB):
                ost = osp.tile([128, C], F32, name="ost", tag="ost", bufs=2)
                for half in range(2):
                    ps = psa.tile([128, 512], F32, name="pps", tag="a",
                                  bufs=2)
                    for m in range(NPAIR):
                        nc.tensor.matmul(
                            ps[:], yts[m][:, tb * 128:(tb + 1) * 128],
                            wpt[:, m, half * 512:(half + 1) * 512],
                            start=(m == 0), stop=(m == NPAIR - 1))
                    nc.vector.tensor_add(
                        ost[:, half * 512:(half + 1) * 512], ps[:],
                        bbt[:, half * 512:(half + 1) * 512])
                nc.sync.dma_start(
                    out=out[tb * 128:(tb + 1) * 128, :], in_=ost[:])

    nc.compile()
    return nc


_NC = None


def _get_nc():
    global _NC
    if _NC is None:
        _NC = build()
    return _NC


def prep_inputs(x, w_attn, w_proj, b_proj):
    x = np.asarray(x, dtype=np.float32)
    w_attn = np.asarray(w_attn, dtype=np.float32)
    w_proj = np.asarray(w_proj, dtype=np.float32)
    b_proj = np.asarray(b_proj, dtype=np.float32)
    ki = np.ascontiguousarray(w_attn[0:C].T).reshape(C, NPAIR, 128)
    qi = np.ascontiguousarray(w_attn[C:2 * C].T).reshape(C, NPAIR, 128)
    wqkv = np.ascontiguousarray(np.concatenate([ki, qi], axis=2))
    wvTv = np.ascontiguousarray(w_attn[2 * C:3 * C].T)
    wpTv = np.ascontiguousarray(w_proj.T)
    bbv = np.broadcast_to(b_proj, (128, C)).copy()
    ii = np.arange(128)
    mk = np.where(ii[None, :] >= ii[:, None], 0.0, NEG).astype(np.float32)
    shared = {"wqk": wqkv, "wvT": wvTv, "wpT": wpTv, "bb": bbv, "maskT": mk}
    in_maps = []
    for b in range(B):
        im = dict(shared)
        im["xT"] = np.ascontiguousarray(x[b].T)
        in_maps.append(im)
    return in_maps


def kernel(x, w_attn, w_proj, b_proj):
    nc = _get_nc()
    in_maps = prep_inputs(x, w_attn, w_proj, b_proj)
    res = run_bass_kernel_spmd(nc, in_maps, core_ids=list(range(NCORES)))
    return np.stack([res.results[b]["out"] for b in range(B)]).astype(np.float32)

